# revision 5
# baseline (speedup 1.0000x reference)
"""Distillation-trainer loss kernel for Trainium2 (8 NeuronCores).

Computes  loss = mean((attn(q,k,v) - attn(q,ck,cv))**2)  for
q:[1,8,1024,128], k/v:[1,8,8192,128], ck/cv:[1,8,1024,128] fp32.

Sharding: one kv-head per core (h axis, 8 heads / 8 cores). Each core
computes its head's squared-error partial sums; the host adds the 8
partials and divides by the element count (the "all-reduce" of the
scalar loss).

Per-core algorithm (head h):
  - load K/Q/CK, transpose to [d, n] layout via PE (fp32 transpose,
    cast to bf16 on the PSUM->SBUF copy).
  - scoresT[n, q] = KT-tile.T @ QT on PE in bf16 (out fp32 PSUM).
  - exp on ACT:  expT = Exp(scoresT * 1/sqrt(d)) -> bf16 SBUF. No
    max-subtraction: scores ~ N(0,1); max over 8k samples < 5, exp
    stays < ~150 which is safely inside fp32/bf16 range.
  - PV: z'[q, 0:128] + S[q] in one accumulation: stationary = expT
    chunk [128n, 128q], moving = v' [128n, 129] where v' has a ones
    column appended; PSUM accumulates over the 64 n-tiles.
  - z = z'[:, :128] * (1 / z'[:, 128]) on DVE; same for compressed;
    (z - zc)^2 row-reduced into per-partition partials.
"""

import numpy as np

import concourse.bass as bass
import concourse.mybir as mybir
import concourse.tile as tile
from concourse import bacc
from concourse.masks import make_identity
from concourse.bass_utils import run_bass_kernel_spmd

F32 = mybir.dt.float32
BF16 = mybir.dt.bfloat16
AF = mybir.ActivationFunctionType
ALU = mybir.AluOpType

B, H, Q, N, NC, D = 1, 8, 1024, 8192, 1024, 128
N_CORES = 8
SCALE = 1.0 / float(np.sqrt(D))

QC = 256                   # q chunk width for the scores moving operand
N_QC = Q // QC             # 4
GT = 4                     # n-tiles per PSUM scores region / ACT call
NT = N // 128              # 64 teacher n-tiles
NCT = NC // 128            # 8 compressed n-tiles


def _emit(nc: bass.Bass, tc: tile.TileContext, qh, kh, vh, ckh, cvh, out_dram):
    ctxs = []

    def pool(**kw):
        p = tc.tile_pool(**kw)
        ctxs.append(p)
        return p.__enter__()

    pconst = pool(name="pconst", bufs=1)
    pstage = pool(name="pstage", bufs=4)
    pex = pool(name="pex", bufs=3)
    psmall = pool(name="psmall", bufs=4)
    ptp = pool(name="ptp", bufs=2, space="PSUM")
    psc = pool(name="psc", bufs=2, space="PSUM")
    pz = pool(name="pz", bufs=1, space="PSUM")

    # ---- persistent SBUF tensors ----
    ident = pconst.tile([128, 128], F32, tag="ident")
    make_identity(nc, ident[:])

    kT = pconst.tile([128, NT, 128], BF16, tag="kT")       # [d, t, n]
    vb = pconst.tile([128, NT, 129], BF16, tag="vb")       # [n, t, d+1]
    qT = pconst.tile([128, Q], BF16, tag="qT")             # [d, q]
    ckT = pconst.tile([128, NCT, 128], BF16, tag="ckT")
    cvb = pconst.tile([128, NCT, 129], BF16, tag="cvb")
    zteach = pconst.tile([128, Q // 128, 128], F32, tag="zteach")  # [q, qt, d]
    accq = pconst.tile([128, Q // 128], F32, tag="accq")

    nc.gpsimd.memset(vb[:, :, 128:129], 1.0)
    nc.gpsimd.memset(cvb[:, :, 128:129], 1.0)

    # ---- load + transpose K, load V (cast fp32 -> bf16) ----
    def load_transposed(src, dst, n_tiles, tag):
        # src [n_tiles*128, 128] fp32 DRAM -> dst [128, n_tiles, 128] bf16 ([d, t, n])
        for g in range(n_tiles // 4):
            stg = pstage.tile([128, 4, 128], F32, tag=tag)
            ap = src[g * 512:(g + 1) * 512, :].rearrange("(i p) d -> p i d", p=128)
            nc.sync.dma_start(out=stg[:], in_=ap)
            for j in range(4):
                t = 4 * g + j
                tp = ptp.tile([128, 128], F32, tag="tp")
                nc.tensor.transpose(tp[:], stg[:, j, :], ident[:])
                nc.vector.tensor_copy(dst[:, t, :], tp[:])

    def load_values(src, dst, n_tiles, tag):
        # src [n_tiles*128, 128] fp32 -> dst [128, n_tiles, 129] bf16 ([n, t, d])
        for g in range(n_tiles // 4):
            stg = pstage.tile([128, 4, 128], F32, tag=tag)
            ap = src[g * 512:(g + 1) * 512, :].rearrange("(i p) d -> p i d", p=128)
            nc.sync.dma_start(out=stg[:], in_=ap)
            nc.vector.tensor_copy(dst[:, 4 * g:4 * g + 4, 0:128], stg[:])

    # q: [1024, 128] -> qT [128, 1024]
    stq = pstage.tile([128, 8, 128], F32, tag="stq")
    nc.sync.dma_start(out=stq[:], in_=qh[:, :].rearrange("(i p) d -> p i d", p=128))
    for i in range(8):
        tp = ptp.tile([128, 128], F32, tag="tp")
        nc.tensor.transpose(tp[:], stq[:, i, :], ident[:])
        nc.vector.tensor_copy(qT[:, i * 128:(i + 1) * 128], tp[:])

    load_transposed(kh, kT, NT, "stk")
    load_values(vh, vb, NT, "stv")
    load_transposed(ckh, ckT, NCT, "stk")
    load_values(cvh, cvb, NCT, "stv")

    # ---- attention + softmax-PV for one q-chunk of 256 ----
    def attend(keysT, vals, n_tiles, qc):
        """Returns (za, zb) PSUM tiles [128, 129] = [unnormalized z | S]."""
        za = pz.tile([128, 129], F32, tag="za")
        zb = pz.tile([128, 129], F32, tag="zb")
        qs = qT[:, qc * QC:(qc + 1) * QC]

        def emit_pv(ex, g):
            for j in range(GT):
                t = GT * g + j
                st = dict(start=(t == 0), stop=(t == n_tiles - 1))
                nc.tensor.matmul(za[:], ex[:, j, 0:128], vals[:, t, :], **st)
                nc.tensor.matmul(zb[:], ex[:, j, 128:256], vals[:, t, :], **st)

        pending = None
        for g in range(n_tiles // GT):
            sp = psc.tile([128, GT, QC], F32, tag="sp")
            for j in range(GT):
                t = GT * g + j
                nc.tensor.matmul(sp[:, j, :], keysT[:, t, :], qs,
                                 start=True, stop=True)
            if pending is not None:
                emit_pv(*pending)
            ex = pex.tile([128, GT, QC], BF16, tag="ex")
            nc.scalar.activation(ex[:], sp[:], AF.Exp, scale=SCALE)
            pending = (ex, g)
        emit_pv(*pending)
        return za, zb

    for qc in range(N_QC):
        # teacher attention for this q chunk
        za, zb = attend(kT, vb, NT, qc)
        for h, zt in ((0, za), (1, zb)):
            qt = qc * 2 + h
            inv = psmall.tile([128, 1], F32, tag="inv")
            nc.vector.reciprocal(inv[:], zt[:, 128:129])
            nc.vector.tensor_scalar_mul(zteach[:, qt, :], zt[:, 0:128], inv[:])

        # compressed attention for this q chunk + MSE partials
        za, zb = attend(ckT, cvb, NCT, qc)
        for h, zt in ((0, za), (1, zb)):
            qt = qc * 2 + h
            inv = psmall.tile([128, 1], F32, tag="inv")
            nc.vector.reciprocal(inv[:], zt[:, 128:129])
            zcn = psmall.tile([128, 128], F32, tag="zcn")
            nc.vector.tensor_scalar_mul(zcn[:], zt[:, 0:128], inv[:])
            d = psmall.tile([128, 128], F32, tag="d")
            nc.vector.tensor_sub(d[:], zcn[:], zteach[:, qt, :])
            d2 = psmall.tile([128, 128], F32, tag="d2")
            nc.vector.tensor_mul(d2[:], d[:], d[:])
            nc.vector.reduce_sum(out=accq[:, qt:qt + 1], in_=d2[:],
                                 axis=mybir.AxisListType.X)

    nc.sync.dma_start(out=out_dram[:], in_=accq[:])

    for p in reversed(ctxs):
        p.__exit__(None, None, None)


_NC_CACHE = None


def build_nc():
    global _NC_CACHE
    if _NC_CACHE is not None:
        return _NC_CACHE
    nc = bacc.Bacc()
    qh = nc.declare_dram_parameter("queries", [Q, D], F32, isOutput=False)
    kh = nc.declare_dram_parameter("keys", [N, D], F32, isOutput=False)
    vh = nc.declare_dram_parameter("values", [N, D], F32, isOutput=False)
    ckh = nc.declare_dram_parameter("c_keys", [NC, D], F32, isOutput=False)
    cvh = nc.declare_dram_parameter("c_values", [NC, D], F32, isOutput=False)
    out = nc.declare_dram_parameter("loss_sums", [128, Q // 128], F32, isOutput=True)
    with tile.TileContext(nc) as tc:
        _emit(nc, tc, qh, kh, vh, ckh, cvh, out)
    nc.compile()
    _NC_CACHE = nc
    return nc


def make_in_maps(queries, keys, values, c_keys, c_values):
    in_maps = []
    for h in range(N_CORES):
        in_maps.append({
            "queries": np.ascontiguousarray(queries[0, h], dtype=np.float32),
            "keys": np.ascontiguousarray(keys[0, h], dtype=np.float32),
            "values": np.ascontiguousarray(values[0, h], dtype=np.float32),
            "c_keys": np.ascontiguousarray(c_keys[0, h], dtype=np.float32),
            "c_values": np.ascontiguousarray(c_values[0, h], dtype=np.float32),
        })
    return in_maps


def run_cores(in_maps, trace=False, **kw):
    nc = build_nc()
    return run_bass_kernel_spmd(nc, in_maps, list(range(N_CORES)),
                                trace=trace, **kw)


def kernel(queries, keys, values, c_keys, c_values):
    res = run_cores(make_in_maps(queries, keys, values, c_keys, c_values))
    total = sum(float(r["loss_sums"].astype(np.float64).sum())
                for r in res.results)
    loss = total / float(B * H * Q * D)
    return np.asarray(loss, dtype=np.float32)
